# revision 27
# baseline (speedup 1.0000x reference)
"""Trainium2 Bass kernel for AttentionWithFP4Projections.

Sharding: tensor-parallel over heads across 8 cores (4 heads each, both
batches). Each core computes q/k/v for its 256 output dims, full causal
attention for its heads, and a partial o_proj (its 256-dim slice of the
contraction); partials are summed on the host (no device reduce).

v3:
 - big matmuls in float32r (1 cyc/row warm; 13-mantissa-bit operands)
   with typed F32R chains end-to-end
 - QKV in 512-token chunks, 4-deep half-chunk operand ring (full double
   buffering) so the PE never waits on HBM between chunks
 - x quant+transpose+allgather pipelined per 128-token piece, all pieces
   up front so no collective ever blocks compute mid-kernel
 - fp4 quant: 3 TT + 5 TS same-ALU-class ops, round-half-down high path
 - exp merged across the two packed heads (one ACTIVATE per [128,1024]
   2-bank PSUM group); causal mask as 0/1 multiply on probs on GpSimd
 - o_proj in bf16 (post-quantization), Wo streamed, run per 512-token
   chunk right after that chunk's attention; bf16 partial output
 - rope DMAs issued on the scalar HWDGE queue to relieve the sync queue
"""
import sys
import types
from contextlib import ExitStack

import numpy as np

# The NTFF profiling hook module is missing in this image; shim it so
# run_bass_kernel_spmd(trace=True) works (used by test.py, harmless here).
if 'antenv.axon_hooks' not in sys.modules:
    _m = types.ModuleType('antenv.axon_hooks')
    _m._hook = None
    _m.set_axon_ntff_profile_hook = lambda h: setattr(_m, '_hook', h)
    _m.get_axon_ntff_profile_hook = lambda: _m._hook
    sys.modules['antenv.axon_hooks'] = _m
    try:
        from trn_agent_boot.trn_boot import _ntff_profile_via_ctypes
        _m._hook = _ntff_profile_via_ctypes('/opt/axon/libaxon_pjrt.so')
    except Exception:
        pass

import concourse.mybir as mybir
import concourse.tile as tile
from concourse import bacc
from concourse import bass_utils
from concourse.masks import make_identity

F32 = mybir.dt.float32
F32R = mybir.dt.float32r
BF16 = mybir.dt.bfloat16
I32 = mybir.dt.int32
ALU = mybir.AluOpType
ACTF = mybir.ActivationFunctionType

NCORES = 8
B, S, HID = 2, 2048, 2048
T = B * S                     # 4096 tokens
NH, HD = 32, 64               # heads, head dim
HPC = NH // NCORES            # 4 heads per core
OD = HPC * HD                 # 256 output dims per core
SPC = S // NCORES             # 256 tokens per batch per core
NCH = S // 512                # 512-token chunks per batch
QW = 256                      # quant working width
MAGIC = 6291456.0             # 1.5*2^22: +/- rounds fp32 to multiples of 0.5


def _quant(nc, sb_tmp, out_ap, in_ap, scale_ap, rs6_ap, P=128):
    """FP4 fake-quant of in_ap [P, QW] -> out_ap, given per-16-block scale
    and rs6 (=6/amax) [P, QW//16].

    y   = x * rs6
    low = magic-round of y to multiples of 0.5   (covers |y| <= 2)
    hi  = (y_bits + 0x1FFFFF) & ~0x3FFFFF        (round-half-down to one
           mantissa bit; covers 2 < |y| <= 6; sign bit untouched)
    q   = |y| > 2 ? hi : low;  out = q * scale
    """
    nb = QW // 16
    y = sb_tmp.tile([128, QW], F32, tag="qt_y", name="qt_y")[:P, :]
    nc.vector.tensor_tensor(
        out=y.rearrange("p (b s) -> p b s", s=16),
        in0=in_ap.rearrange("p (b s) -> p b s", s=16),
        in1=rs6_ap.unsqueeze(2).broadcast_to([P, nb, 16]),
        op=ALU.mult)
    low = sb_tmp.tile([128, QW], F32, tag="qt_l", name="qt_l")[:P, :]
    nc.vector.tensor_scalar(out=low, in0=y,
                            scalar1=MAGIC, scalar2=MAGIC,
                            op0=ALU.add, op1=ALU.subtract)
    hi = sb_tmp.tile([128, QW], I32, tag="qt_h", name="qt_h")[:P, :]
    nc.vector.tensor_scalar(out=hi, in0=y.bitcast(I32),
                            scalar1=0x1FFFFF, scalar2=None, op0=ALU.add)
    nc.vector.tensor_scalar(out=hi, in0=hi,
                            scalar1=-4194304, scalar2=None,
                            op0=ALU.bitwise_and)
    # |y| then float-compare vs 2.0, in place in y's buffer (y dead after)
    nc.vector.tensor_scalar(out=y.bitcast(I32), in0=y.bitcast(I32),
                            scalar1=0x7FFFFFFF, scalar2=None,
                            op0=ALU.bitwise_and)
    pred = y.bitcast(I32)
    nc.vector.tensor_scalar(out=pred, in0=y,
                            scalar1=2.0, scalar2=None, op0=ALU.is_gt)
    nc.vector.copy_predicated(low, pred, hi.bitcast(F32))
    nc.vector.tensor_tensor(
        out=out_ap.rearrange("p (b s) -> p b s", s=16),
        in0=low.rearrange("p (b s) -> p b s", s=16),
        in1=scale_ap.unsqueeze(2).broadcast_to([P, nb, 16]),
        op=ALU.mult)


def _amax_scales(nc, sb_tmp, in_ap, P=128, want_scale=True):
    """Returns (scale, rs6, amax) [P, QW//16] tiles for fp4 quant of
    in_ap [P, QW]. amax is clamped in place; rs6 shares rcp's buffer."""
    nb = QW // 16
    amax = sb_tmp.tile([128, 16], F32, tag="am", name="am")[:P, :nb]
    nc.vector.tensor_reduce(amax, in_ap.rearrange("p (b s) -> p b s", s=16),
                            axis=mybir.AxisListType.X, op=ALU.max,
                            apply_absolute_value=True)
    nc.vector.tensor_scalar_max(amax, amax, 1e-30)
    rcp = sb_tmp.tile([128, 16], F32, tag="rc", name="rc")[:P, :nb]
    nc.vector.reciprocal(rcp, amax)
    rs6 = rcp
    nc.vector.tensor_scalar_mul(rs6, rcp, 6.0)
    scale = None
    if want_scale:
        scale = sb_tmp.tile([128, 16], F32, tag="sc", name="sc")[:P, :nb]
        nc.vector.tensor_scalar_mul(scale, amax, 1.0 / 6.0)
    return scale, rs6, amax


def build():
    nc = bacc.Bacc("TRN2", target_bir_lowering=False, debug=False,
                   num_devices=NCORES)
    x_d = nc.dram_tensor("x", [2 * SPC, HID], F32,
                         kind="ExternalInput").ap()  # this core's tokens
    # allgather split in two 128-token pieces per batch for overlap
    xg_in = [[nc.dram_tensor(f"xg_in{b}_{pc}", [HID, 128], F32R)
              for pc in range(2)] for b in range(B)]
    xg_out = [[nc.dram_tensor(f"xg_out{b}_{pc}", [NCORES, HID, 128], F32R,
                              addr_space="Shared") for pc in range(2)]
              for b in range(B)]
    wq_d = nc.dram_tensor("wqT", [16, 128, OD], F32R,
                          kind="ExternalInput").ap()
    wk_d = nc.dram_tensor("wkT", [16, 128, OD], F32R,
                          kind="ExternalInput").ap()
    wv_d = nc.dram_tensor("wvT", [16, 128, OD], F32R,
                          kind="ExternalInput").ap()
    wo_d = nc.dram_tensor("woT", [2, 128, HID], BF16,
                          kind="ExternalInput").ap()
    cos_d = nc.dram_tensor("cosT", [128, S], F32R, kind="ExternalInput").ap()
    sin_d = nc.dram_tensor("sinTs", [128, S], F32R,
                           kind="ExternalInput").ap()
    mask_d = nc.dram_tensor("masks", [128, 128], F32R,
                            kind="ExternalInput").ap()
    out_d = nc.dram_tensor("partialT", [HID, T], BF16,
                           kind="ExternalOutput").ap()

    with tile.TileContext(nc) as tc, ExitStack() as ctx:
        sb_w = ctx.enter_context(tc.tile_pool(name="sb_w", bufs=1))
        sb_tmp = ctx.enter_context(tc.tile_pool(name="sb_tmp", bufs=1))
        sb_io = ctx.enter_context(tc.tile_pool(name="sb_io", bufs=2))
        sb_att = ctx.enter_context(tc.tile_pool(name="sb_att", bufs=1))
        sb_x = ctx.enter_context(tc.tile_pool(name="sb_x", bufs=4))
        sb_pt = ctx.enter_context(tc.tile_pool(name="sb_pt", bufs=2))
        ps_sc = ctx.enter_context(
            tc.tile_pool(name="ps_sc", bufs=2, space="PSUM"))
        ps_ot = ctx.enter_context(
            tc.tile_pool(name="ps_ot", bufs=1, space="PSUM"))
        ps_mm = ctx.enter_context(
            tc.tile_pool(name="ps_mm", bufs=2, space="PSUM"))

        ident = sb_w.tile([128, 128], F32)
        make_identity(nc, ident[:])
        masks = sb_w.tile([128, 128], F32R)
        nc.sync.dma_start(masks[:], mask_d)

        # --------- weights: pre-quantized + transposed on host ---------
        wT = {}
        for nm, wd in (("q", wq_d), ("k", wk_d), ("v", wv_d)):
            wt = sb_w.tile([128, 16 * OD], F32R, name=f"w{nm}T")
            wT[nm] = wt
            nc.sync.dma_start(wt[:].rearrange("p (a t) -> p a t", a=16),
                              wd.rearrange("a p t -> p a t"))

        # ---------------- x quant + transpose + allgather ----------------
        _xqTl = []

        def x_prep_tile(b, ti):
            """Quantize+transpose 128 tokens, then allgather that piece."""
            if not _xqTl:
                _xqTl.append(sb_att.tile([128, 16 * 128], F32R, tag="xqT",
                                         name="xqTl"))
            xqTl = _xqTl[0]
            xrow = sb_x.tile([128, HID], F32, tag="xh", name="xrow")
            nc.sync.dma_start(
                xrow[:],
                x_d[b * SPC + ti * 128: b * SPC + (ti + 1) * 128, :])
            for off in range(0, HID, QW):
                seg = xrow[:, off:off + QW]
                scale, rs6, _ = _amax_scales(nc, sb_tmp, seg)
                _quant(nc, sb_tmp, seg, seg, scale, rs6)
            for i in range(16):
                pt = ps_mm.tile([128, 512], F32, tag="ps_mm",
                                name="pt")[:, 0:128]
                nc.tensor.transpose(
                    pt, xrow[:, i * 128:(i + 1) * 128], ident[:])
                dst = xqTl[:, i * 128:(i + 1) * 128]
                if i % 2 == 0:
                    nc.scalar.copy(dst, pt)
                else:
                    nc.vector.tensor_copy(dst, pt)
            nc.sync.dma_start(
                xg_in[b][ti].ap().rearrange("(a p) t -> p a t", p=128),
                xqTl[:].rearrange("p (a t) -> p a t", a=16))
            nc.gpsimd.collective_compute(
                "AllGather", ALU.bypass,
                replica_groups=[list(range(NCORES))],
                ins=[xg_in[b][ti].ap()], outs=[xg_out[b][ti].ap()])

        for ti in range(SPC // 128):
            x_prep_tile(0, ti)

        # persistent per-batch buffers
        qT = [sb_att.tile([128, S], F32R, name=f"qT{m}") for m in range(2)]
        kT = [sb_att.tile([128, S], F32R, name=f"kT{m}") for m in range(2)]
        vE = [sb_att.tile([128, 16 * 65], F32R, name=f"vE{h}")
              for h in range(HPC)]
        oqT = [sb_att.tile([128, S], BF16, name=f"oqT{m}") for m in range(2)]
        rsum = sb_tmp.tile([128, 16], F32, name="rsum")
        rraw = sb_tmp.tile([128, 16], F32, name="rraw")

        # ones columns of vE: written once, never overwritten after
        # (memset has no f32r flavor -> memset f32 then copy-round)
        ones = sb_tmp.tile([128, 16], F32, name="ones")
        nc.vector.memset(ones[:], 1.0)
        for h in range(HPC):
            nc.vector.tensor_copy(
                vE[h][:].rearrange("p (k c) -> p k c", c=65)[:, :, 64:65],
                ones[:].unsqueeze(2))

        def rope_piece(dst, m, ch, cosT, sinT):
            """RoPE in place on dst[m][:, ch*512:(ch+1)*512]."""
            c0 = ch * 512
            sh = sb_io.tile([128, 512], F32R, tag="rope_sh", bufs=1)
            for hh in range(2):
                p0 = hh * 64
                nc.scalar.dma_start(
                    sh[p0:p0 + 32, :],
                    dst[m][p0 + 32:p0 + 64, c0:c0 + 512])
                nc.scalar.dma_start(
                    sh[p0 + 32:p0 + 64, :],
                    dst[m][p0:p0 + 32, c0:c0 + 512])
            nc.gpsimd.tensor_tensor(out=sh[:], in0=sh[:],
                                    in1=sinT[:], op=ALU.mult)
            # in place: the cos multiply write waits on the shuffle reads
            nc.vector.tensor_tensor(
                out=dst[m][:, c0:c0 + 512], in0=dst[m][:, c0:c0 + 512],
                in1=cosT[:], op=ALU.mult)
            nc.vector.tensor_tensor(
                out=dst[m][:, c0:c0 + 512], in0=dst[m][:, c0:c0 + 512],
                in1=sh[:], op=ALU.add)

        for b in range(B):
            t0 = b * S

            # -------- projections over 512-token chunks --------
            for ch in range(NCH):
                cc0 = ch * 512
                cosT = sb_io.tile([128, 512], F32R, tag="rope_c", bufs=1)
                sinT = sb_io.tile([128, 512], F32R, tag="rope_s", bufs=1)
                nc.scalar.dma_start(cosT[:], cos_d[:, cc0:cc0 + 512])
                nc.scalar.dma_start(sinT[:], sin_d[:, cc0:cc0 + 512])
                # two half-chunk moving-operand buffers (8 a-tiles each),
                # 4-deep ring = full double buffering across chunks
                xh = []
                for h2 in range(2):
                    xt = sb_x.tile([128, 8 * 512], F32R, tag="xh",
                                   name="xh")
                    xh.append(xt)
                    for cc in range(2):
                        for pc in range(2):
                            nc.sync.dma_start(
                                xt[:].rearrange("p (a t) -> p a t", a=8)
                                [:, :, cc * 256 + pc * 128:
                                 cc * 256 + (pc + 1) * 128],
                                xg_out[b][pc].ap()[2 * ch + cc].rearrange(
                                    "(a p) t -> p a t", p=128)
                                [:, 8 * h2:8 * h2 + 8, :])
                for nm in ("k", "q", "v"):
                    for m in range(2):
                        pj = ps_mm.tile([128, 512], F32, tag="ps_mm",
                                        name="pj")
                        for i in range(16):
                            nc.tensor.matmul(
                                pj[:],
                                wT[nm][:, i * OD + m * 128:
                                       i * OD + (m + 1) * 128],
                                xh[i // 8][:, (i % 8) * 512:
                                           (i % 8 + 1) * 512],
                                start=(i == 0), stop=(i == 15))
                        if nm == "v":
                            vsb = sb_io.tile([128, 512], F32, tag="vsb",
                                             bufs=2)
                            nc.scalar.copy(vsb[:], pj[:])
                            for hh in range(2):
                                h_ = m * 2 + hh
                                for kt in range(4):
                                    ptv = ps_mm.tile([128, 512], F32,
                                                     tag="ps_mm",
                                                     name="ptv")[:, 0:128]
                                    nc.tensor.transpose(
                                        ptv[:, 0:64],
                                        vsb[hh * 64:(hh + 1) * 64,
                                            kt * 128:(kt + 1) * 128],
                                        ident[hh * 64:(hh + 1) * 64,
                                              hh * 64:(hh + 1) * 64])
                                    ktile = ch * 4 + kt
                                    nc.vector.tensor_copy(
                                        vE[h_][:, ktile * 65:
                                               ktile * 65 + 64],
                                        ptv[:, 0:64])
                        else:
                            dst = qT if nm == "q" else kT
                            if m == 0:
                                nc.scalar.copy(dst[m][:, cc0:cc0 + 512],
                                               pj[:])
                            else:
                                nc.vector.tensor_copy(
                                    dst[m][:, cc0:cc0 + 512], pj[:])
                            rope_piece(dst, m, ch, cosT, sinT)

            # batch-1 x prep + gathers run during batch-0 attention,
            # after the chunk ring is drained (no slot stealing)
            if b == 0:
                for ti in range(SPC // 128):
                    x_prep_tile(1, ti)

            # -------- attention + per-chunk o_proj --------
            for qc in range(4):
                onat = sb_att.tile([128, 1024], F32, tag="xqT",
                                   name="onat")
                for m in range(2):
                    oT = ps_ot.tile([65, 1024], F32, tag="ps_oT",
                                    name="ps_oT")
                    for kblk in range(4 * qc + 4):
                        qs = max(qc * 512, kblk * 128)
                        w = (qc + 1) * 512 - qs
                        off = qs - qc * 512
                        diag = kblk >= 4 * qc
                        sc2 = ps_sc.tile([128, 1024], F32, tag="sc2",
                                         name="sc2")
                        for hh in range(2):
                            p0 = hh * 64
                            nc.tensor.matmul(
                                sc2[:, hh * 512: hh * 512 + w],
                                kT[m][p0:p0 + 64,
                                      kblk * 128:(kblk + 1) * 128],
                                qT[m][p0:p0 + 64, qs:(qc + 1) * 512],
                                start=True, stop=True)
                        pT = sb_pt.tile([128, 1024], F32R, tag="pT",
                                        name="pT")
                        if w == 512:
                            nc.scalar.activation(pT[:], sc2[:],
                                                 ACTF.Exp, scale=0.125)
                        else:
                            nc.scalar.activation(
                                pT[:].rearrange("p (h w) -> p h w", h=2)
                                [:, :, 0:w],
                                sc2[:].rearrange("p (h w) -> p h w", h=2)
                                [:, :, 0:w],
                                ACTF.Exp, scale=0.125)
                        if diag:
                            for hh in range(2):
                                nc.gpsimd.tensor_tensor(
                                    out=pT[:, hh * 512: hh * 512 + 128],
                                    in0=pT[:, hh * 512: hh * 512 + 128],
                                    in1=masks[:], op=ALU.mult)
                        for hh in range(2):
                            h_ = m * 2 + hh
                            nc.tensor.matmul(
                                oT[:, hh * 512 + off: hh * 512 + off + w],
                                vE[h_][:, kblk * 65:(kblk + 1) * 65],
                                pT[:, hh * 512: hh * 512 + w],
                                start=(kblk == 0),
                                stop=(kblk == 4 * qc + 3),
                                skip_group_check=(kblk == 4 * qc + 3
                                                  and off != 0))
                    # evacuate oT: transpose to natural + collect row sums
                    # (shares the pT ring slot - same size, phase-disjoint)
                    osb = sb_pt.tile([128, 1024], F32, tag="pT",
                                     name="osb")
                    nc.scalar.copy(osb[0:65, :], oT[:])
                    for hh in range(2):
                        h_ = m * 2 + hh
                        for tt in range(4):
                            ptn = ps_mm.tile([128, 512], F32, tag="ps_mm",
                                             name="ptn")[:, 0:128]
                            nc.tensor.transpose(
                                ptn[:, 0:65],
                                osb[0:65, hh * 512 + tt * 128:
                                    hh * 512 + (tt + 1) * 128],
                                ident[0:65, 0:65])
                            nc.vector.tensor_copy(
                                onat[:, tt * 256 + h_ * 64:
                                     tt * 256 + (h_ + 1) * 64],
                                ptn[:, 0:64])
                            nc.vector.tensor_copy(
                                rraw[:, tt * 4 + h_: tt * 4 + h_ + 1],
                                ptn[:, 64:65])
                nc.vector.reciprocal(rsum[:], rraw[:])
                # quantize [128, 256] quarters (one tt group), fold 1/sum
                for q4 in range(4):
                    seg = onat[:, q4 * 256:(q4 + 1) * 256]
                    _, rs6, amax = _amax_scales(nc, sb_tmp, seg,
                                                want_scale=False)
                    sct = sb_tmp.tile([128, 16], F32, tag="sc",
                                      name="sct")
                    nc.vector.tensor_tensor(
                        out=sct[:].rearrange("p (h s) -> p h s", s=4),
                        in0=amax.rearrange("p (h s) -> p h s", s=4),
                        in1=rsum[:, q4 * 4:(q4 + 1) * 4]
                        .unsqueeze(2).broadcast_to([128, 4, 4]),
                        op=ALU.mult)
                    nc.vector.tensor_scalar_mul(sct[:], sct[:], 1.0 / 6.0)
                    # oq aliases the quant hi-buffer (dead by the final
                    # scale multiply)
                    oq = sb_tmp.tile([128, QW], F32, tag="qt_h",
                                     name="oq")
                    _quant(nc, sb_tmp, oq[:], seg, sct[:], rs6)
                    for j in range(2):
                        ptq = ps_mm.tile([128, 512], F32, tag="ps_mm",
                                         name="ptq")[:, 0:128]
                        nc.tensor.transpose(
                            ptq, oq[:, j * 128:(j + 1) * 128], ident[:])
                        nc.vector.tensor_copy(
                            oqT[j][:, qc * 512 + q4 * 128:
                                   qc * 512 + (q4 + 1) * 128],
                            ptq)
                # ---- o_proj for this 512-token chunk ----
                for mo in range(16):
                    wos = sb_io.tile([128, 256], BF16, tag="wos",
                                     name="wos", bufs=2)
                    nc.sync.dma_start(
                        wos[:].rearrange("p (i o) -> p i o", i=2),
                        wo_d.rearrange("i p o -> p i o")
                        [:, :, mo * 128:(mo + 1) * 128])
                    po = ps_mm.tile([128, 512], F32, tag="ps_mm", name="po")
                    for i in range(2):
                        nc.tensor.matmul(
                            po[:],
                            wos[:, i * 128:(i + 1) * 128],
                            oqT[i][:, qc * 512:(qc + 1) * 512],
                            start=(i == 0), stop=(i == 1))
                    posb = sb_io.tile([128, 512], BF16, tag="posb",
                                      name="posb")
                    if mo % 2 == 0:
                        nc.scalar.copy(posb[:], po[:])
                    else:
                        nc.vector.tensor_copy(posb[:], po[:])
                    nc.sync.dma_start(
                        out_d[mo * 128:(mo + 1) * 128,
                              t0 + qc * 512:t0 + (qc + 1) * 512],
                        posb[:])

    nc.compile()
    return nc


def _np_quant(x):
    """Host fp4 fake-quant, matching the device implementation."""
    sh = x.shape
    xb = x.reshape(sh[:-1] + (sh[-1] // 16, 16)).astype(np.float32)
    amax = np.max(np.abs(xb), axis=-1, keepdims=True).astype(np.float32)
    amax_c = np.maximum(amax, np.float32(1e-30))
    rcp = (np.float32(1.0) / amax_c).astype(np.float32)
    rs6 = (rcp * np.float32(6.0)).astype(np.float32)
    scale = (amax * np.float32(1.0 / 6.0)).astype(np.float32)
    y = (xb * rs6).astype(np.float32)
    yi = y.view(np.int32)
    hi = ((yi + np.int32(0x1FFFFF)) & np.int32(-4194304)).view(np.float32)
    M32 = np.float32(MAGIC)
    low = ((y + M32).astype(np.float32) - M32).astype(np.float32)
    q = np.where(np.abs(y) > np.float32(2.0), hi, low)
    return (q * scale).astype(np.float32).reshape(sh)


_HOST_CACHE = {}


def _host_tables():
    if _HOST_CACHE:
        return _HOST_CACHE
    D = HD
    inv = (1.0 / (10000.0 ** (np.arange(0, D, 2, dtype=np.float32)
                              / np.float32(D)))).astype(np.float32)
    fr = (np.arange(S, dtype=np.float32)[:, None] * inv[None, :]).astype(
        np.float32)
    cos = np.concatenate([np.cos(fr), np.cos(fr)], -1).astype(np.float32)
    sin = np.concatenate([np.sin(fr), np.sin(fr)], -1).astype(np.float32)
    sgn = np.where(np.arange(D) < D // 2, np.float32(-1.0), np.float32(1.0))
    cosT = np.tile(cos.T, (2, 1)).astype(np.float32)          # [128, S]
    sinTs = np.tile((sin * sgn[None, :]).T, (2, 1)).astype(np.float32)
    # 0/1 triangle for post-exp masking, sT layout: k-row kk allows q >= kk
    masks = np.zeros((128, 128), np.float32)
    for kk in range(128):
        masks[kk, kk:] = 1.0
    _HOST_CACHE.update(cosT=cosT, sinTs=sinTs, masks=masks)
    return _HOST_CACHE


_NC_CACHE = []


def make_in_maps(hidden_states, Wq, Wk, Wv, Wo):
    bf16 = mybir.dt.np(BF16)
    tabs = _host_tables()
    xf = hidden_states.reshape(T, HID)
    wq_q = _np_quant(np.asarray(Wq, np.float32))
    wk_q = _np_quant(np.asarray(Wk, np.float32))
    wv_q = _np_quant(np.asarray(Wv, np.float32))
    wo_q = _np_quant(np.asarray(Wo, np.float32))
    in_maps = []
    for c in range(NCORES):
        sl = slice(c * OD, (c + 1) * OD)
        xl = np.concatenate([xf[c * SPC:(c + 1) * SPC],
                             xf[S + c * SPC: S + (c + 1) * SPC]], axis=0)
        wqT = np.ascontiguousarray(
            wq_q[sl, :].T.reshape(16, 128, OD), np.float32)
        wkT = np.ascontiguousarray(
            wk_q[sl, :].T.reshape(16, 128, OD), np.float32)
        wvT = np.ascontiguousarray(
            wv_q[sl, :].T.reshape(16, 128, OD), np.float32)
        woTc = np.ascontiguousarray(
            wo_q[:, sl].T.reshape(2, 128, HID)).astype(bf16)
        in_maps.append(dict(
            x=np.ascontiguousarray(xl, np.float32),
            wqT=wqT, wkT=wkT, wvT=wvT, woT=woTc,
            cosT=tabs['cosT'], sinTs=tabs['sinTs'], masks=tabs['masks'],
        ))
    return in_maps


def kernel(hidden_states, Wq, Wk, Wv, Wo):
    in_maps = make_in_maps(hidden_states, Wq, Wk, Wv, Wo)
    if not _NC_CACHE:
        _NC_CACHE.append(build())
    nc = _NC_CACHE[0]
    res = bass_utils.run_bass_kernel_spmd(nc, in_maps,
                                          core_ids=list(range(NCORES)))
    total = np.zeros((HID, T), np.float32)
    for r in res.results:
        total += np.asarray(r["partialT"], np.float32)
    return np.ascontiguousarray(total.T.reshape(B, S, HID))


if __name__ == "__main__":
    d = np.load('/root/problem/inputs.npz')
    out = kernel(d['hidden_states'], d['Wq'], d['Wk'], d['Wv'], d['Wo'])
    ref = np.load('/root/problem/ref_out.npy')
    rel2 = np.linalg.norm(out - ref) / np.linalg.norm(ref)
    print(f"relL2={rel2:.3e} absmax={np.abs(out - ref).max():.3e}")


# revision 28
# speedup vs baseline: 1.0237x; 1.0237x over previous
"""Trainium2 Bass kernel for AttentionWithFP4Projections.

Sharding: tensor-parallel over heads across 8 cores (4 heads each, both
batches). Each core computes q/k/v for its 256 output dims, full causal
attention for its heads, and a partial o_proj (its 256-dim slice of the
contraction); partials are summed on the host (no device reduce).

v3:
 - big matmuls in float32r (1 cyc/row warm; 13-mantissa-bit operands)
   with typed F32R chains end-to-end
 - QKV in 512-token chunks, 4-deep half-chunk operand ring (full double
   buffering) so the PE never waits on HBM between chunks
 - x quant+transpose+allgather pipelined per 128-token piece, all pieces
   up front so no collective ever blocks compute mid-kernel
 - fp4 quant: 3 TT + 5 TS same-ALU-class ops, round-half-down high path
 - exp merged across the two packed heads (one ACTIVATE per [128,1024]
   2-bank PSUM group); causal mask as 0/1 multiply on probs on GpSimd
 - o_proj in bf16 (post-quantization), Wo streamed, run per 512-token
   chunk right after that chunk's attention; bf16 partial output
 - rope DMAs issued on the scalar HWDGE queue to relieve the sync queue
"""
import sys
import types
from contextlib import ExitStack

import numpy as np

# The NTFF profiling hook module is missing in this image; shim it so
# run_bass_kernel_spmd(trace=True) works (used by test.py, harmless here).
if 'antenv.axon_hooks' not in sys.modules:
    _m = types.ModuleType('antenv.axon_hooks')
    _m._hook = None
    _m.set_axon_ntff_profile_hook = lambda h: setattr(_m, '_hook', h)
    _m.get_axon_ntff_profile_hook = lambda: _m._hook
    sys.modules['antenv.axon_hooks'] = _m
    try:
        from trn_agent_boot.trn_boot import _ntff_profile_via_ctypes
        _m._hook = _ntff_profile_via_ctypes('/opt/axon/libaxon_pjrt.so')
    except Exception:
        pass

import concourse.mybir as mybir
import concourse.tile as tile
from concourse import bacc
from concourse import bass_utils
from concourse.masks import make_identity

F32 = mybir.dt.float32
F32R = mybir.dt.float32r
BF16 = mybir.dt.bfloat16
I32 = mybir.dt.int32
ALU = mybir.AluOpType
ACTF = mybir.ActivationFunctionType

NCORES = 8
B, S, HID = 2, 2048, 2048
T = B * S                     # 4096 tokens
NH, HD = 32, 64               # heads, head dim
HPC = NH // NCORES            # 4 heads per core
OD = HPC * HD                 # 256 output dims per core
SPC = S // NCORES             # 256 tokens per batch per core
NCH = S // 512                # 512-token chunks per batch
QW = 256                      # quant working width
MAGIC = 6291456.0             # 1.5*2^22: +/- rounds fp32 to multiples of 0.5


def _quant(nc, sb_tmp, out_ap, in_ap, scale_ap, rs6_ap, P=128):
    """FP4 fake-quant of in_ap [P, QW] -> out_ap, given per-16-block scale
    and rs6 (=6/amax) [P, QW//16].

    y   = x * rs6
    low = magic-round of y to multiples of 0.5   (covers |y| <= 2)
    hi  = (y_bits + 0x1FFFFF) & ~0x3FFFFF        (round-half-down to one
           mantissa bit; covers 2 < |y| <= 6; sign bit untouched)
    q   = |y| > 2 ? hi : low;  out = q * scale
    """
    nb = QW // 16
    y = sb_tmp.tile([128, QW], F32, tag="qt_y", name="qt_y")[:P, :]
    nc.vector.tensor_tensor(
        out=y.rearrange("p (b s) -> p b s", s=16),
        in0=in_ap.rearrange("p (b s) -> p b s", s=16),
        in1=rs6_ap.unsqueeze(2).broadcast_to([P, nb, 16]),
        op=ALU.mult)
    low = sb_tmp.tile([128, QW], F32, tag="qt_l", name="qt_l")[:P, :]
    nc.vector.tensor_scalar(out=low, in0=y,
                            scalar1=MAGIC, scalar2=MAGIC,
                            op0=ALU.add, op1=ALU.subtract)
    hi = sb_tmp.tile([128, QW], I32, tag="qt_h", name="qt_h")[:P, :]
    nc.vector.tensor_scalar(out=hi, in0=y.bitcast(I32),
                            scalar1=0x1FFFFF, scalar2=None, op0=ALU.add)
    nc.vector.tensor_scalar(out=hi, in0=hi,
                            scalar1=-4194304, scalar2=None,
                            op0=ALU.bitwise_and)
    # |y| then float-compare vs 2.0, in place in y's buffer (y dead after)
    nc.vector.tensor_scalar(out=y.bitcast(I32), in0=y.bitcast(I32),
                            scalar1=0x7FFFFFFF, scalar2=None,
                            op0=ALU.bitwise_and)
    pred = y.bitcast(I32)
    nc.vector.tensor_scalar(out=pred, in0=y,
                            scalar1=2.0, scalar2=None, op0=ALU.is_gt)
    nc.vector.copy_predicated(low, pred, hi.bitcast(F32))
    nc.vector.tensor_tensor(
        out=out_ap.rearrange("p (b s) -> p b s", s=16),
        in0=low.rearrange("p (b s) -> p b s", s=16),
        in1=scale_ap.unsqueeze(2).broadcast_to([P, nb, 16]),
        op=ALU.mult)


def _amax_scales(nc, sb_tmp, in_ap, P=128, want_scale=True):
    """Returns (scale, rs6, amax) [P, QW//16] tiles for fp4 quant of
    in_ap [P, QW]. amax is clamped in place; rs6 shares rcp's buffer."""
    nb = QW // 16
    amax = sb_tmp.tile([128, 16], F32, tag="am", name="am")[:P, :nb]
    nc.vector.tensor_reduce(amax, in_ap.rearrange("p (b s) -> p b s", s=16),
                            axis=mybir.AxisListType.X, op=ALU.max,
                            apply_absolute_value=True)
    nc.vector.tensor_scalar_max(amax, amax, 1e-30)
    rcp = sb_tmp.tile([128, 16], F32, tag="rc", name="rc")[:P, :nb]
    nc.vector.reciprocal(rcp, amax)
    rs6 = rcp
    nc.vector.tensor_scalar_mul(rs6, rcp, 6.0)
    scale = None
    if want_scale:
        scale = sb_tmp.tile([128, 16], F32, tag="sc", name="sc")[:P, :nb]
        nc.vector.tensor_scalar_mul(scale, amax, 1.0 / 6.0)
    return scale, rs6, amax


def build():
    nc = bacc.Bacc("TRN2", target_bir_lowering=False, debug=False,
                   num_devices=NCORES)
    x_d = nc.dram_tensor("x", [2 * SPC, HID], F32,
                         kind="ExternalInput").ap()  # this core's tokens
    # allgather split in two 128-token pieces per batch for overlap
    xg_in = [[nc.dram_tensor(f"xg_in{b}_{pc}", [HID, 128], F32R)
              for pc in range(2)] for b in range(B)]
    xg_out = [[nc.dram_tensor(f"xg_out{b}_{pc}", [NCORES, HID, 128], F32R,
                              addr_space="Shared") for pc in range(2)]
              for b in range(B)]
    wq_d = nc.dram_tensor("wqT", [16, 128, OD], F32R,
                          kind="ExternalInput").ap()
    wk_d = nc.dram_tensor("wkT", [16, 128, OD], F32R,
                          kind="ExternalInput").ap()
    wv_d = nc.dram_tensor("wvT", [16, 128, OD], F32R,
                          kind="ExternalInput").ap()
    wo_d = nc.dram_tensor("woT", [2, 128, HID], BF16,
                          kind="ExternalInput").ap()
    cos_d = nc.dram_tensor("cosT", [128, S], F32R, kind="ExternalInput").ap()
    sin_d = nc.dram_tensor("sinTs", [128, S], F32R,
                           kind="ExternalInput").ap()
    mask_d = nc.dram_tensor("masks", [128, 128], F32R,
                            kind="ExternalInput").ap()
    out_d = nc.dram_tensor("partialT", [HID, T], BF16,
                           kind="ExternalOutput").ap()

    with tile.TileContext(nc) as tc, ExitStack() as ctx:
        sb_w = ctx.enter_context(tc.tile_pool(name="sb_w", bufs=1))
        sb_tmp = ctx.enter_context(tc.tile_pool(name="sb_tmp", bufs=1))
        sb_io = ctx.enter_context(tc.tile_pool(name="sb_io", bufs=2))
        sb_att = ctx.enter_context(tc.tile_pool(name="sb_att", bufs=1))
        sb_x = ctx.enter_context(tc.tile_pool(name="sb_x", bufs=4))
        sb_pt = ctx.enter_context(tc.tile_pool(name="sb_pt", bufs=2))
        ps_sc = ctx.enter_context(
            tc.tile_pool(name="ps_sc", bufs=2, space="PSUM"))
        ps_ot = ctx.enter_context(
            tc.tile_pool(name="ps_ot", bufs=1, space="PSUM"))
        ps_mm = ctx.enter_context(
            tc.tile_pool(name="ps_mm", bufs=2, space="PSUM"))

        ident = sb_w.tile([128, 128], F32)
        make_identity(nc, ident[:])
        masks = sb_w.tile([128, 128], F32R)
        nc.sync.dma_start(masks[:], mask_d)

        # --------- weights: pre-quantized + transposed on host ---------
        wT = {}
        for nm, wd in (("q", wq_d), ("k", wk_d), ("v", wv_d)):
            wt = sb_w.tile([128, 16 * OD], F32R, name=f"w{nm}T")
            wT[nm] = wt
            nc.sync.dma_start(wt[:].rearrange("p (a t) -> p a t", a=16),
                              wd.rearrange("a p t -> p a t"))

        # ---------------- x quant + transpose + allgather ----------------
        _xqTl = []

        def x_prep_tile(b, ti):
            """Quantize+transpose 128 tokens, then allgather that piece."""
            if not _xqTl:
                _xqTl.append(sb_att.tile([128, 16 * 128], F32R, tag="xqT",
                                         name="xqTl"))
            xqTl = _xqTl[0]
            xrow = sb_x.tile([128, HID], F32, tag="xh", name="xrow")
            nc.sync.dma_start(
                xrow[:],
                x_d[b * SPC + ti * 128: b * SPC + (ti + 1) * 128, :])
            for off in range(0, HID, QW):
                seg = xrow[:, off:off + QW]
                scale, rs6, _ = _amax_scales(nc, sb_tmp, seg)
                _quant(nc, sb_tmp, seg, seg, scale, rs6)
            for i in range(16):
                pt = ps_mm.tile([128, 512], F32, tag="ps_mm",
                                name="pt")[:, 0:128]
                nc.tensor.transpose(
                    pt, xrow[:, i * 128:(i + 1) * 128], ident[:])
                dst = xqTl[:, i * 128:(i + 1) * 128]
                if i % 2 == 0:
                    nc.scalar.copy(dst, pt)
                else:
                    nc.vector.tensor_copy(dst, pt)
            nc.sync.dma_start(
                xg_in[b][ti].ap().rearrange("(a p) t -> p a t", p=128),
                xqTl[:].rearrange("p (a t) -> p a t", a=16))
            nc.gpsimd.collective_compute(
                "AllGather", ALU.bypass,
                replica_groups=[list(range(NCORES))],
                ins=[xg_in[b][ti].ap()], outs=[xg_out[b][ti].ap()])

        for b_ in range(B):
            for ti in range(SPC // 128):
                x_prep_tile(b_, ti)

        # persistent per-batch buffers
        qT = [sb_att.tile([128, S], F32R, name=f"qT{m}") for m in range(2)]
        kT = [sb_att.tile([128, S], F32R, name=f"kT{m}") for m in range(2)]
        vE = [sb_att.tile([128, 16 * 65], F32R, name=f"vE{h}")
              for h in range(HPC)]
        oqT = [sb_att.tile([128, S], BF16, name=f"oqT{m}") for m in range(2)]
        rsum = sb_tmp.tile([128, 16], F32, name="rsum")
        rraw = sb_tmp.tile([128, 16], F32, name="rraw")

        # ones columns of vE: written once, never overwritten after
        # (memset has no f32r flavor -> memset f32 then copy-round)
        ones = sb_tmp.tile([128, 16], F32, name="ones")
        nc.vector.memset(ones[:], 1.0)
        for h in range(HPC):
            nc.vector.tensor_copy(
                vE[h][:].rearrange("p (k c) -> p k c", c=65)[:, :, 64:65],
                ones[:].unsqueeze(2))

        def rope_piece(dst, m, ch, cosT, sinT):
            """RoPE in place on dst[m][:, ch*512:(ch+1)*512]."""
            c0 = ch * 512
            sh = sb_io.tile([128, 512], F32R, tag="rope_sh", bufs=1)
            for hh in range(2):
                p0 = hh * 64
                nc.scalar.dma_start(
                    sh[p0:p0 + 32, :],
                    dst[m][p0 + 32:p0 + 64, c0:c0 + 512])
                nc.scalar.dma_start(
                    sh[p0 + 32:p0 + 64, :],
                    dst[m][p0:p0 + 32, c0:c0 + 512])
            nc.gpsimd.tensor_tensor(out=sh[:], in0=sh[:],
                                    in1=sinT[:], op=ALU.mult)
            # in place: the cos multiply write waits on the shuffle reads
            nc.vector.tensor_tensor(
                out=dst[m][:, c0:c0 + 512], in0=dst[m][:, c0:c0 + 512],
                in1=cosT[:], op=ALU.mult)
            nc.vector.tensor_tensor(
                out=dst[m][:, c0:c0 + 512], in0=dst[m][:, c0:c0 + 512],
                in1=sh[:], op=ALU.add)

        for b in range(B):
            t0 = b * S

            # -------- projections over 512-token chunks --------
            for ch in range(NCH):
                cc0 = ch * 512
                cosT = sb_io.tile([128, 512], F32R, tag="rope_c", bufs=1)
                sinT = sb_io.tile([128, 512], F32R, tag="rope_s", bufs=1)
                nc.scalar.dma_start(cosT[:], cos_d[:, cc0:cc0 + 512])
                nc.scalar.dma_start(sinT[:], sin_d[:, cc0:cc0 + 512])
                # two half-chunk moving-operand buffers (8 a-tiles each),
                # 4-deep ring = full double buffering across chunks
                xh = []
                for h2 in range(2):
                    xt = sb_x.tile([128, 8 * 512], F32R, tag="xh",
                                   name="xh")
                    xh.append(xt)
                    for cc in range(2):
                        for pc in range(2):
                            nc.sync.dma_start(
                                xt[:].rearrange("p (a t) -> p a t", a=8)
                                [:, :, cc * 256 + pc * 128:
                                 cc * 256 + (pc + 1) * 128],
                                xg_out[b][pc].ap()[2 * ch + cc].rearrange(
                                    "(a p) t -> p a t", p=128)
                                [:, 8 * h2:8 * h2 + 8, :])
                for nm in ("k", "q", "v"):
                    for m in range(2):
                        pj = ps_mm.tile([128, 512], F32, tag="ps_mm",
                                        name="pj")
                        for i in range(16):
                            nc.tensor.matmul(
                                pj[:],
                                wT[nm][:, i * OD + m * 128:
                                       i * OD + (m + 1) * 128],
                                xh[i // 8][:, (i % 8) * 512:
                                           (i % 8 + 1) * 512],
                                start=(i == 0), stop=(i == 15))
                        if nm == "v":
                            vsb = sb_io.tile([128, 512], F32, tag="vsb",
                                             bufs=2)
                            nc.scalar.copy(vsb[:], pj[:])
                            for hh in range(2):
                                h_ = m * 2 + hh
                                for kt in range(4):
                                    ptv = ps_mm.tile([128, 512], F32,
                                                     tag="ps_mm",
                                                     name="ptv")[:, 0:128]
                                    nc.tensor.transpose(
                                        ptv[:, 0:64],
                                        vsb[hh * 64:(hh + 1) * 64,
                                            kt * 128:(kt + 1) * 128],
                                        ident[hh * 64:(hh + 1) * 64,
                                              hh * 64:(hh + 1) * 64])
                                    ktile = ch * 4 + kt
                                    nc.vector.tensor_copy(
                                        vE[h_][:, ktile * 65:
                                               ktile * 65 + 64],
                                        ptv[:, 0:64])
                        else:
                            dst = qT if nm == "q" else kT
                            if m == 0:
                                nc.scalar.copy(dst[m][:, cc0:cc0 + 512],
                                               pj[:])
                            else:
                                nc.vector.tensor_copy(
                                    dst[m][:, cc0:cc0 + 512], pj[:])
                            rope_piece(dst, m, ch, cosT, sinT)

            # -------- attention + per-chunk o_proj --------
            for qc in range(4):
                onat = sb_att.tile([128, 1024], F32, tag="xqT",
                                   name="onat")
                for m in range(2):
                    oT = ps_ot.tile([65, 1024], F32, tag="ps_oT",
                                    name="ps_oT")
                    for kblk in range(4 * qc + 4):
                        qs = max(qc * 512, kblk * 128)
                        w = (qc + 1) * 512 - qs
                        off = qs - qc * 512
                        diag = kblk >= 4 * qc
                        sc2 = ps_sc.tile([128, 1024], F32, tag="sc2",
                                         name="sc2")
                        for hh in range(2):
                            p0 = hh * 64
                            nc.tensor.matmul(
                                sc2[:, hh * 512: hh * 512 + w],
                                kT[m][p0:p0 + 64,
                                      kblk * 128:(kblk + 1) * 128],
                                qT[m][p0:p0 + 64, qs:(qc + 1) * 512],
                                start=True, stop=True)
                        pT = sb_pt.tile([128, 1024], F32R, tag="pT",
                                        name="pT")
                        if w == 512:
                            nc.scalar.activation(pT[:], sc2[:],
                                                 ACTF.Exp, scale=0.125)
                        else:
                            nc.scalar.activation(
                                pT[:].rearrange("p (h w) -> p h w", h=2)
                                [:, :, 0:w],
                                sc2[:].rearrange("p (h w) -> p h w", h=2)
                                [:, :, 0:w],
                                ACTF.Exp, scale=0.125)
                        if diag:
                            for hh in range(2):
                                nc.gpsimd.tensor_tensor(
                                    out=pT[:, hh * 512: hh * 512 + 128],
                                    in0=pT[:, hh * 512: hh * 512 + 128],
                                    in1=masks[:], op=ALU.mult)
                        for hh in range(2):
                            h_ = m * 2 + hh
                            nc.tensor.matmul(
                                oT[:, hh * 512 + off: hh * 512 + off + w],
                                vE[h_][:, kblk * 65:(kblk + 1) * 65],
                                pT[:, hh * 512: hh * 512 + w],
                                start=(kblk == 0),
                                stop=(kblk == 4 * qc + 3),
                                skip_group_check=(kblk == 4 * qc + 3
                                                  and off != 0))
                    # evacuate oT: transpose to natural + collect row sums
                    # (shares the pT ring slot - same size, phase-disjoint)
                    osb = sb_pt.tile([128, 1024], F32, tag="pT",
                                     name="osb")
                    nc.scalar.copy(osb[0:65, :], oT[:])
                    for hh in range(2):
                        h_ = m * 2 + hh
                        for tt in range(4):
                            ptn = ps_mm.tile([128, 512], F32, tag="ps_mm",
                                             name="ptn")[:, 0:128]
                            nc.tensor.transpose(
                                ptn[:, 0:65],
                                osb[0:65, hh * 512 + tt * 128:
                                    hh * 512 + (tt + 1) * 128],
                                ident[0:65, 0:65])
                            nc.vector.tensor_copy(
                                onat[:, tt * 256 + h_ * 64:
                                     tt * 256 + (h_ + 1) * 64],
                                ptn[:, 0:64])
                            nc.vector.tensor_copy(
                                rraw[:, tt * 4 + h_: tt * 4 + h_ + 1],
                                ptn[:, 64:65])
                nc.vector.reciprocal(rsum[:], rraw[:])
                # quantize [128, 256] quarters (one tt group), fold 1/sum
                for q4 in range(4):
                    seg = onat[:, q4 * 256:(q4 + 1) * 256]
                    _, rs6, amax = _amax_scales(nc, sb_tmp, seg,
                                                want_scale=False)
                    sct = sb_tmp.tile([128, 16], F32, tag="sc",
                                      name="sct")
                    nc.vector.tensor_tensor(
                        out=sct[:].rearrange("p (h s) -> p h s", s=4),
                        in0=amax.rearrange("p (h s) -> p h s", s=4),
                        in1=rsum[:, q4 * 4:(q4 + 1) * 4]
                        .unsqueeze(2).broadcast_to([128, 4, 4]),
                        op=ALU.mult)
                    nc.vector.tensor_scalar_mul(sct[:], sct[:], 1.0 / 6.0)
                    # oq aliases the quant hi-buffer (dead by the final
                    # scale multiply)
                    oq = sb_tmp.tile([128, QW], F32, tag="qt_h",
                                     name="oq")
                    _quant(nc, sb_tmp, oq[:], seg, sct[:], rs6)
                    for j in range(2):
                        ptq = ps_mm.tile([128, 512], F32, tag="ps_mm",
                                         name="ptq")[:, 0:128]
                        nc.tensor.transpose(
                            ptq, oq[:, j * 128:(j + 1) * 128], ident[:])
                        nc.vector.tensor_copy(
                            oqT[j][:, qc * 512 + q4 * 128:
                                   qc * 512 + (q4 + 1) * 128],
                            ptq)
                # ---- o_proj for this 512-token chunk ----
                for mo in range(16):
                    wos = sb_io.tile([128, 256], BF16, tag="wos",
                                     name="wos", bufs=2)
                    nc.sync.dma_start(
                        wos[:].rearrange("p (i o) -> p i o", i=2),
                        wo_d.rearrange("i p o -> p i o")
                        [:, :, mo * 128:(mo + 1) * 128])
                    po = ps_mm.tile([128, 512], F32, tag="ps_mm", name="po")
                    for i in range(2):
                        nc.tensor.matmul(
                            po[:],
                            wos[:, i * 128:(i + 1) * 128],
                            oqT[i][:, qc * 512:(qc + 1) * 512],
                            start=(i == 0), stop=(i == 1))
                    posb = sb_io.tile([128, 512], BF16, tag="posb",
                                      name="posb")
                    if mo % 2 == 0:
                        nc.scalar.copy(posb[:], po[:])
                    else:
                        nc.vector.tensor_copy(posb[:], po[:])
                    nc.sync.dma_start(
                        out_d[mo * 128:(mo + 1) * 128,
                              t0 + qc * 512:t0 + (qc + 1) * 512],
                        posb[:])

    nc.compile()
    return nc


def _np_quant(x):
    """Host fp4 fake-quant, matching the device implementation."""
    sh = x.shape
    xb = x.reshape(sh[:-1] + (sh[-1] // 16, 16)).astype(np.float32)
    amax = np.max(np.abs(xb), axis=-1, keepdims=True).astype(np.float32)
    amax_c = np.maximum(amax, np.float32(1e-30))
    rcp = (np.float32(1.0) / amax_c).astype(np.float32)
    rs6 = (rcp * np.float32(6.0)).astype(np.float32)
    scale = (amax * np.float32(1.0 / 6.0)).astype(np.float32)
    y = (xb * rs6).astype(np.float32)
    yi = y.view(np.int32)
    hi = ((yi + np.int32(0x1FFFFF)) & np.int32(-4194304)).view(np.float32)
    M32 = np.float32(MAGIC)
    low = ((y + M32).astype(np.float32) - M32).astype(np.float32)
    q = np.where(np.abs(y) > np.float32(2.0), hi, low)
    return (q * scale).astype(np.float32).reshape(sh)


_HOST_CACHE = {}


def _host_tables():
    if _HOST_CACHE:
        return _HOST_CACHE
    D = HD
    inv = (1.0 / (10000.0 ** (np.arange(0, D, 2, dtype=np.float32)
                              / np.float32(D)))).astype(np.float32)
    fr = (np.arange(S, dtype=np.float32)[:, None] * inv[None, :]).astype(
        np.float32)
    cos = np.concatenate([np.cos(fr), np.cos(fr)], -1).astype(np.float32)
    sin = np.concatenate([np.sin(fr), np.sin(fr)], -1).astype(np.float32)
    sgn = np.where(np.arange(D) < D // 2, np.float32(-1.0), np.float32(1.0))
    cosT = np.tile(cos.T, (2, 1)).astype(np.float32)          # [128, S]
    sinTs = np.tile((sin * sgn[None, :]).T, (2, 1)).astype(np.float32)
    # 0/1 triangle for post-exp masking, sT layout: k-row kk allows q >= kk
    masks = np.zeros((128, 128), np.float32)
    for kk in range(128):
        masks[kk, kk:] = 1.0
    _HOST_CACHE.update(cosT=cosT, sinTs=sinTs, masks=masks)
    return _HOST_CACHE


_NC_CACHE = []


def make_in_maps(hidden_states, Wq, Wk, Wv, Wo):
    bf16 = mybir.dt.np(BF16)
    tabs = _host_tables()
    xf = hidden_states.reshape(T, HID)
    wq_q = _np_quant(np.asarray(Wq, np.float32))
    wk_q = _np_quant(np.asarray(Wk, np.float32))
    wv_q = _np_quant(np.asarray(Wv, np.float32))
    wo_q = _np_quant(np.asarray(Wo, np.float32))
    in_maps = []
    for c in range(NCORES):
        sl = slice(c * OD, (c + 1) * OD)
        xl = np.concatenate([xf[c * SPC:(c + 1) * SPC],
                             xf[S + c * SPC: S + (c + 1) * SPC]], axis=0)
        wqT = np.ascontiguousarray(
            wq_q[sl, :].T.reshape(16, 128, OD), np.float32)
        wkT = np.ascontiguousarray(
            wk_q[sl, :].T.reshape(16, 128, OD), np.float32)
        wvT = np.ascontiguousarray(
            wv_q[sl, :].T.reshape(16, 128, OD), np.float32)
        woTc = np.ascontiguousarray(
            wo_q[:, sl].T.reshape(2, 128, HID)).astype(bf16)
        in_maps.append(dict(
            x=np.ascontiguousarray(xl, np.float32),
            wqT=wqT, wkT=wkT, wvT=wvT, woT=woTc,
            cosT=tabs['cosT'], sinTs=tabs['sinTs'], masks=tabs['masks'],
        ))
    return in_maps


def kernel(hidden_states, Wq, Wk, Wv, Wo):
    in_maps = make_in_maps(hidden_states, Wq, Wk, Wv, Wo)
    if not _NC_CACHE:
        _NC_CACHE.append(build())
    nc = _NC_CACHE[0]
    res = bass_utils.run_bass_kernel_spmd(nc, in_maps,
                                          core_ids=list(range(NCORES)))
    total = np.zeros((HID, T), np.float32)
    for r in res.results:
        total += np.asarray(r["partialT"], np.float32)
    return np.ascontiguousarray(total.T.reshape(B, S, HID))


if __name__ == "__main__":
    d = np.load('/root/problem/inputs.npz')
    out = kernel(d['hidden_states'], d['Wq'], d['Wk'], d['Wv'], d['Wo'])
    ref = np.load('/root/problem/ref_out.npy')
    rel2 = np.linalg.norm(out - ref) / np.linalg.norm(ref)
    print(f"relL2={rel2:.3e} absmax={np.abs(out - ref).max():.3e}")
